# revision 11
# baseline (speedup 1.0000x reference)
"""Trainium2 Bass kernel for nn_ColorLoss: mean CIEDE2000 over RGB images.

Sharding: pure data parallel over batch - 16 images, 8 cores, 2 images/core.
Each core computes per-partition partial sums of deltaE; host reduces.

v3 (validated numerically in proto.py / CoreSim, rel err ~4e-5):
- No-branch sRGB gamma exp(2.4*ln((c+.055)/1.055)), batched per tensor.
- No-branch cbrt; Lab scales and a global 1/64 rescale folded into Exp
  biases so the a,b,C pipeline runs in fp16 (DVE 2x/4x perf modes).
- Hue without arctan/sin tables: cos h, sin h from the normalized hue
  bisector; T via Chebyshev in (cos h, sin h); dtheta Gaussian via
  z = K*(1-cos(h-275deg))/2; sin(2*dtheta) small-angle poly.
- Single activation table set (ln/exp/square): sqrt = exp(0.5*ln),
  rsqrt = exp(-0.5*ln)  ->  no ACT_TABLE_LOAD churn.
- tC,tH divisions eliminated via common denominator D=SC*SH; deltaE =
  64*sqrt(N)/D folded into the final Exp(0.5*lnN - 0.5*lnD2 + ln64).
- Software-pipelined: chunk k+1 head (DMA/gamma/Lab/C-chain) is emitted
  before chunk k tail (hue/T/Rc/final) so the in-order scalar queue never
  starves the vector engine; cross-boundary tiles live in a bufs=2 pool.
- GpSimd gets only off-critical-path work (XYZ combos, L-chain side).
"""
import sys

sys.path.insert(0, '/opt/trn_rl_repo')

import math

import numpy as np

import concourse.bacc as bacc
import concourse.mybir as mybir
import concourse.tile as tile

AF = mybir.ActivationFunctionType
OP = mybir.AluOpType
F32 = mybir.dt.float32
F16 = mybir.dt.float16

B, C, H, W = 16, 3, 512, 512
NCORE = 8
IPC = B // NCORE
PLANE = H * W
PF = PLANE // 128
FCH = 1024
NCH_IMG = PF // FCH
NCHUNK = IPC * NCH_IMG

M = [[0.412453, 0.357580, 0.180423],
     [0.212671, 0.715160, 0.072169],
     [0.019334, 0.119193, 0.950227]]
WHITE = [0.95047, 1.0, 1.08883]
SCL = 64.0
KP7 = (25.0 / SCL) ** 7
K_G = (360.0 / (25.0 * math.pi)) ** 2
KL = 116.0 * SCL / 500.0

B_GAMMA = 0.055 / 1.055
B_LN500 = math.log(500.0 / SCL)
B_LN200 = math.log(200.0 / SCL)
B_Q = -66.0
B_S20 = 20.0
B_GAUSS = math.log(math.pi / 3.0)
B_TINY = 1e-12
B_LNSCL = math.log(SCL)
B_NN = 1e-7
ACT_BIASES = (B_GAMMA, B_LN500, B_LN200, B_Q, B_S20, B_GAUSS, B_TINY,
              B_LNSCL, B_NN, KP7)

C30, S30 = math.cos(math.radians(30)), math.sin(math.radians(30))
C6, S6 = math.cos(math.radians(6)), math.sin(math.radians(6))
C63, S63 = math.cos(math.radians(63)), math.sin(math.radians(63))
C275 = math.cos(math.radians(275))
S275 = math.sin(math.radians(275))

_NC_CACHE = {}


class _Ops:
    """Thin emit helpers bound to one Bacc + pools."""

    def __init__(self, nc, wk, xp):
        self.nc = nc
        self.wk = wk
        self.xp = xp
        self.P, self.F = 128, FCH

    def ts(self, tag, src, s1, op0, s2=None, op1=None, dt=F16, pool=None):
        t = (pool or self.wk).tile([self.P, self.F], dt, tag=tag)
        return self.tsip(t, src, s1, op0, s2, op1)

    def tsip(self, dst, src, s1, op0, s2=None, op1=None):
        if s2 is None:
            self.nc.vector.tensor_scalar(out=dst[:], in0=src[:],
                                         scalar1=float(s1), scalar2=None,
                                         op0=op0)
        else:
            self.nc.vector.tensor_scalar(out=dst[:], in0=src[:],
                                         scalar1=float(s1),
                                         scalar2=float(s2), op0=op0, op1=op1)
        return dst

    def tt(self, tag, a, b, op, dt=F16, pool=None):
        t = (pool or self.wk).tile([self.P, self.F], dt, tag=tag)
        self.nc.vector.tensor_tensor(out=t[:], in0=a[:], in1=b[:], op=op)
        return t

    def ttip(self, dst, a, b, op):
        self.nc.vector.tensor_tensor(out=dst[:], in0=a[:], in1=b[:], op=op)
        return dst

    def gt(self, tag, a, b, op, dt=F16, pool=None):
        t = (pool or self.wk).tile([self.P, self.F], dt, tag=tag)
        self.nc.gpsimd.tensor_tensor(out=t[:], in0=a[:], in1=b[:], op=op)
        return t

    def gtip(self, dst, a, b, op):
        self.nc.gpsimd.tensor_tensor(out=dst[:], in0=a[:], in1=b[:], op=op)
        return dst

    def sact(self, tag, src, fn, scale=1.0, bias=0.0, dt=F16, accum=None,
             pool=None):
        t = (pool or self.wk).tile([self.P, self.F], dt, tag=tag)
        self.nc.scalar.activation(t[:], src[:], fn, scale=float(scale),
                                  bias=bias, accum_out=accum)
        return t

    def sact_ip(self, dst, fn, scale=1.0, bias=0.0):
        self.nc.scalar.activation(dst[:], dst[:], fn, scale=float(scale),
                                  bias=bias)
        return dst

    def sqrt2(self, tag, src, bias=0.0, dt=F16, pool=None):
        t = self.sact(tag, src, AF.Ln, bias=(bias if bias else B_TINY),
                      dt=dt, pool=pool)
        return self.sact_ip(t, AF.Exp, scale=0.5)

    def rcp(self, tag, src):
        t = self.wk.tile([self.P, self.F], F32, tag=tag)
        self.nc.vector.reciprocal_approx_fast(out=t[:], in_=src[:])
        return t


def _emit_head(o, iop, t_out, t_lab, img, ci):
    """DMA + gamma + Lab + L-chain + C-chain for one chunk.

    Returns the cross-boundary state (tiles in the bufs=2 xp pool).
    """
    nc, P, F = o.nc, o.P, o.F
    S = nc.scalar
    sl = slice(ci * FCH, (ci + 1) * FCH)

    in3 = []
    for t_i, t_dram in enumerate((t_lab, t_out)):
        t3 = iop.tile([P, 3 * F], F32, tag=f"in3_{t_i}")
        for ch in range(3):
            view = t_dram[img, ch].rearrange("(p n) w -> p (n w)", p=128)
            nc.sync.dma_start(t3[:, ch * F:(ch + 1) * F], view[:, sl])
        in3.append(t3)

    fys, aa, bb = [], [], []
    for i in range(2):
        S.activation(in3[i][:], in3[i][:], AF.Ln, scale=1.0 / 1.055,
                     bias=B_GAMMA)
        lin = o.wk.tile([P, 3 * F], F16, tag=f"lin{i}")
        S.activation(lin[:], in3[i][:], AF.Exp, scale=2.4)
        lr = lin[:, 0 * F:1 * F]
        lg = lin[:, 1 * F:2 * F]
        lb = lin[:, 2 * F:3 * F]
        lnt = []
        for k in range(3):
            m0, m1, m2 = M[k]
            w1 = o.ts("sA", lg, m1 / m0, OP.mult)
            ta = o.gt("sB", lr, w1, OP.add)
            w2 = o.ts("sA", lb, m2 / m0, OP.mult)
            tk = o.gtip(ta, ta, w2, OP.add)
            lnt.append(o.sact(f"lnt{k}", tk, AF.Ln, scale=m0 / WHITE[k],
                              dt=F32))
        fx = o.sact("h0", lnt[0], AF.Exp, scale=1 / 3, bias=B_LN500)
        fy = o.sact(f"fys{i}", lnt[1], AF.Exp, scale=1 / 3, bias=B_LN500)
        fz = o.sact("h1", lnt[2], AF.Exp, scale=1 / 3, bias=B_LN200)
        aa.append(o.tt(f"a{i}", fx, fy, OP.subtract, pool=o.xp))
        fy2 = o.ts("h2", fy, 0.4, OP.mult)
        bb.append(o.tt(f"b{i}", fy2, fz, OP.subtract, pool=o.xp))
        fys.append(fy)
    fys1, fys2 = fys
    a1, a2 = aa
    b1, b2 = bb

    # L chain (off critical path; mostly GpSimd + acts)
    lsum = o.gt("h0", fys1, fys2, OP.add)
    dl = o.gt("h1", fys2, fys1, OP.subtract)
    q = o.sact("g0", lsum, AF.Square, scale=KL / 2, bias=B_Q, dt=F32)
    s20l = o.sact("g1", q, AF.Ln, bias=B_S20, dt=F32)
    rs20 = o.sact("g2", s20l, AF.Exp, scale=-0.5, dt=F32)
    wq = o.gtip(q, q, rs20, OP.mult)
    SL = o.sact("g1", wq, AF.Identity, scale=0.015, bias=1.0, dt=F32)
    rSL = o.rcp("g3", SL)
    tl = o.gt("h3", dl, rSL, OP.mult)
    tlsq = o.gt("tlsq", tl, tl, OP.mult, pool=o.xp)

    # C chain
    b1sq = o.gt("b1sq", b1, b1, OP.mult)
    b2sq = o.gt("b2sq", b2, b2, OP.mult)
    a1sq = o.gt("h0", a1, a1, OP.mult)
    a2sq = o.gt("h1", a2, a2, OP.mult)
    c1sq = o.gt("h2", a1sq, b1sq, OP.add)
    c2sq = o.gt("h3", a2sq, b2sq, OP.add)
    C1 = o.sqrt2("h4", c1sq)
    C2 = o.sqrt2("h5", c2sq)
    cb = o.tt("h0", C1, C2, OP.add)
    cbh = o.ts("h1", cb, 0.5, OP.mult)
    u = o.tt("h2", cbh, cbh, OP.mult)
    u2 = o.tt("h3", u, u, OP.mult)
    u3 = o.tt("h4", u2, u, OP.mult)
    c7 = o.tt("h5", u3, cbh, OP.mult)
    den = o.sact("g2", c7, AF.Identity, bias=KP7, dt=F32)
    rden = o.rcp("g3", den)
    rat = o.ttip(c7, c7, rden, OP.mult)
    sr = o.sqrt2("h6", rat)
    opg = o.ts("h7", sr, -0.5, OP.mult, 1.5, OP.add)
    a1p = o.tt("a1p", a1, opg, OP.mult, pool=o.xp)
    a2p = o.tt("a2p", a2, opg, OP.mult, pool=o.xp)
    a1psq = o.gt("h0", a1p, a1p, OP.mult)
    a2psq = o.gt("h1", a2p, a2p, OP.mult)
    c1psq = o.gt("h2", a1psq, b1sq, OP.add)
    c2psq = o.gt("h3", a2psq, b2sq, OP.add)
    C1p = o.sqrt2("C1p", c1psq, pool=o.xp)
    C2p = o.sqrt2("C2p", c2psq, pool=o.xp)
    dC = o.tt("dC", C2p, C1p, OP.subtract, pool=o.xp)
    tsum = o.tt("tsum", C1p, C2p, OP.add, pool=o.xp)

    return dict(b1=b1, b2=b2, a1p=a1p, a2p=a2p, C1p=C1p, C2p=C2p,
                dC=dC, tsum=tsum, tlsq=tlsq)


def _emit_tail(o, st, acc, chunk):
    """Hue, T, Rc, gaussian, final assembly + accumulation for one chunk."""
    b1, b2 = st["b1"], st["b2"]
    a1p, a2p = st["a1p"], st["a2p"]
    C1p, C2p = st["C1p"], st["C2p"]
    dC, tsum, tlsq = st["dC"], st["tsum"], st["tlsq"]

    # dH (sqrt half-angle form, explicit sign)
    pa = o.tt("t0", a1p, a2p, OP.mult)
    pb = o.tt("t1", b1, b2, OP.mult)
    hm = o.ttip(pb, pa, pb, OP.add)
    prodC = o.tt("t2", C1p, C2p, OP.mult)
    dot = o.tt("t0", prodC, hm, OP.subtract)
    dpos = o.ts("t1", dot, 0.0, OP.max, 2.0, OP.mult)
    dH = o.sqrt2("t3", dpos)
    cr1 = o.tt("t0", b2, a1p, OP.mult)
    cr2 = o.tt("t1", a2p, b1, OP.mult)
    crs = o.ttip(cr1, cr1, cr2, OP.subtract)
    sg2 = o.ts("t1", crs, 0.0, OP.is_gt, 2.0, OP.mult)
    sgm = o.tsip(sg2, sg2, -1.0, OP.add)
    dHs = o.tt("dHs", dH, sgm, OP.mult)

    # hue bisector -> cos h, sin h
    ny1 = o.tt("t0", b1, C2p, OP.mult)
    ny2 = o.tt("t1", b2, C1p, OP.mult)
    ny = o.ttip(ny1, ny1, ny2, OP.add)
    nx1 = o.tt("t1", a1p, C2p, OP.mult)
    nx2 = o.tt("t2", a2p, C1p, OP.mult)
    nx = o.ttip(nx1, nx1, nx2, OP.add)
    nsq = o.tt("t2", nx, nx, OP.mult)
    msq = o.tt("t3", ny, ny, OP.mult)
    nn = o.ttip(nsq, nsq, msq, OP.add)
    nnl = o.sact("k0", nn, AF.Ln, bias=B_NN, dt=F32)
    rN = o.sact("t9", nnl, AF.Exp, scale=-0.5)
    ch = o.tt("ch", nx, rN, OP.mult)
    sh = o.tt("sh", ny, rN, OP.mult)

    # T = P1(c2) + c*P2(c2) + s*P3(c2) + s*c*P4(c2)   (c2 = cos^2 h)
    c2t = o.tt("t0", ch, ch, OP.mult)
    c4t = o.tt("t1", c2t, c2t, OP.mult)
    sc_ = o.tt("t2", sh, ch, OP.mult)
    P2t = o.ts("t3", c2t, 1.273008, OP.mult, -1.101980, OP.add)
    cP2 = o.ttip(P2t, P2t, ch, OP.mult)
    P3t = o.ts("t4", c2t, -0.133788, OP.mult, -0.051553, OP.add)
    sP3 = o.ttip(P3t, P3t, sh, OP.mult)
    P4t = o.ts("t5", c2t, -1.425610, OP.mult, 0.712805, OP.add)
    scP4 = o.ttip(P4t, P4t, sc_, OP.mult)
    P1a = o.ts("t6", c2t, 1.206384, OP.mult, 0.669202, OP.add)
    P1b = o.ts("t7", c4t, -0.726384, OP.mult)
    s12 = o.ttip(cP2, cP2, sP3, OP.add)
    s34 = o.ttip(scP4, scP4, P1a, OP.add)
    s56 = o.ttip(s12, s12, s34, OP.add)
    T = o.tt("T", s56, P1b, OP.add)

    # SC/SH, common-denominator products
    ttn = o.tt("t0", tsum, T, OP.mult)
    SH = o.ts("t1", ttn, 0.015 * SCL / 2, OP.mult, 1.0, OP.add)
    SC = o.ts("t2", tsum, 0.045 * SCL / 2, OP.mult, 1.0, OP.add)
    A = o.tt("t3", dC, SH, OP.mult)
    Bt = o.tt("t4", dHs, SC, OP.mult)
    D = o.tt("t5", SC, SH, OP.mult)
    D2 = o.ttip(D, D, D, OP.mult)
    A2 = o.tt("t6", A, A, OP.mult)
    B2 = o.tt("t7", Bt, Bt, OP.mult)
    AB = o.ttip(A, A, Bt, OP.mult)
    s1t = o.ttip(A2, A2, B2, OP.add)

    # Rc
    cbp = o.ts("t8", tsum, 0.5, OP.mult)
    up = o.tt("t1", cbp, cbp, OP.mult)
    up2 = o.tt("t2", up, up, OP.mult)
    up3 = o.tt("t4", up2, up, OP.mult)
    c7p = o.ttip(up2, up3, cbp, OP.mult)
    denp = o.sact("k0", c7p, AF.Identity, bias=KP7, dt=F32)
    rdp = o.rcp("k1", denp)
    ratp = o.ttip(c7p, c7p, rdp, OP.mult)
    srp = o.sqrt2("t0", ratp)

    # gaussian dtheta: z = K/2 - K/2*cos(h-275)
    za = o.ts("t4", ch, -K_G / 2 * C275, OP.mult, K_G / 2, OP.add)
    zb = o.ts("t7", sh, -K_G / 2 * S275, OP.mult)
    z = o.ttip(za, za, zb, OP.add)
    xg = o.sact("t7", z, AF.Exp, scale=-1.0, bias=B_GAUSS)
    xs2 = o.tt("t4", xg, xg, OP.mult)
    wco = o.tsip(xs2, xs2, -1.0 / 6.0, OP.mult, 1.0, OP.add)
    sn = o.ttip(xg, xg, wco, OP.mult)

    # final: N = A^2+B^2-2*srp*sn*A*B + tL^2*D^2; dE = 64*sqrt(N)/D
    rtc = o.ttip(srp, srp, sn, OP.mult)
    crt = o.ttip(rtc, AB, rtc, OP.mult)
    s2t = o.tsip(crt, crt, 2.0, OP.mult)
    Fi = o.ttip(s1t, s1t, s2t, OP.subtract)
    fa = o.ts("t1", tlsq, (KL / SCL) ** 2, OP.mult)
    faD = o.ttip(fa, fa, D2, OP.mult)
    Fi2 = o.ttip(Fi, Fi, faD, OP.add)
    Fp = o.tsip(Fi2, Fi2, 0.0, OP.max)
    lnN = o.sact("k0", Fp, AF.Ln, bias=B_TINY, dt=F32)
    lnD2 = o.sact("k1", D2, AF.Ln, dt=F32)
    df = o.ttip(lnN, lnN, lnD2, OP.subtract)
    o.sact("k1", df, AF.Exp, scale=0.5, bias=B_LNSCL, dt=F32,
           accum=acc[:, chunk:chunk + 1])


def _restrict_act_tables(arch):
    """Single activation table set: only natural_log_exp_and_others keeps
    {Ln, Exp, Square}; every other set is emptied so the load inserter can
    never pick them (dict order = act_func_set_id, so entries must stay)."""
    from concourse.hw_specs import get_activation_tables
    tabs = get_activation_tables(arch)
    for name, fset in tabs.items():
        if name == "natural_log_exp_and_others":
            fset.intersection_update({AF.Ln, AF.Exp, AF.Square,
                                      AF.Identity})
        else:
            fset.clear()


def _build():
    nc = bacc.Bacc("TRN2", target_bir_lowering=False, debug=False)
    _restrict_act_tables(nc.m.arch)
    t_out = nc.declare_dram_parameter("outputs", [IPC, C, H, W], F32,
                                      isOutput=False)
    t_lab = nc.declare_dram_parameter("labels", [IPC, C, H, W], F32,
                                      isOutput=False)
    t_part = nc.declare_dram_parameter("partial", [128, NCHUNK], F32,
                                       isOutput=True)
    for i, v in enumerate(ACT_BIASES):
        t = nc.alloc_sbuf_tensor(f"constx{i}", [128, 1], F32)
        nc.gpsimd.memset(t.ap(), v)
        nc.const_aps.aps[(F32, v)] = t.ap()
    nc.all_engine_barrier()
    with tile.TileContext(nc) as tc:
        with tc.tile_pool(name="io", bufs=2) as iop, \
             tc.tile_pool(name="wk", bufs=1) as wk, \
             tc.tile_pool(name="xp", bufs=2) as xp, \
             tc.tile_pool(name="accp", bufs=1) as accp:
            acc = accp.tile([128, NCHUNK], F32, tag="acc")
            o = _Ops(nc, wk, xp)
            states = []
            for img in range(IPC):
                for ci in range(NCH_IMG):
                    states.append(_emit_head(o, iop, t_out, t_lab, img, ci))
                    k = len(states) - 1
                    if k >= 1:
                        _emit_tail(o, states[k - 1], acc, k - 1)
            _emit_tail(o, states[-1], acc, NCHUNK - 1)
            nc.sync.dma_start(t_part[:, :], acc[:, :])
    nc.compile()
    return nc


def get_nc():
    if "nc" not in _NC_CACHE:
        _NC_CACHE["nc"] = _build()
    return _NC_CACHE["nc"]


def kernel(outputs: np.ndarray, labels: np.ndarray) -> np.ndarray:
    from concourse.bass_utils import run_bass_kernel_spmd

    outputs = np.ascontiguousarray(outputs, dtype=np.float32)
    labels = np.ascontiguousarray(labels, dtype=np.float32)
    nc = get_nc()
    in_maps = [{"outputs": outputs[i * IPC:(i + 1) * IPC],
                "labels": labels[i * IPC:(i + 1) * IPC]}
               for i in range(NCORE)]
    res = run_bass_kernel_spmd(nc, in_maps, core_ids=list(range(NCORE)))
    total = 0.0
    for r in res.results:
        total += r["partial"].astype(np.float64).sum()
    return np.float32(total / (B * H * W))


if __name__ == "__main__":
    rng = np.random.default_rng(0)
    o = rng.uniform(0, 1, (B, C, H, W)).astype(np.float32)
    l = rng.uniform(0, 1, (B, C, H, W)).astype(np.float32)
    print(kernel(o, l))
